# revision 18
# baseline (speedup 1.0000x reference)
"""MLA (DeepSeek-style) attention block on 8 Trainium2 NeuronCores.

Sharding:
  phase 1 (token-parallel, 8 x 512 tokens): LoRA-A down-projections + rmsnorm
    + k_pe rope; small AllGather of the kv latents (576 dims); q up-projection
    for ALL 16 heads on the token side + rope, shipped to head shards via two
    shard-aligned AllToAlls (pe+even-nope first, odd-nope second).
  phase 2 (head-parallel, 2 heads x 2 batches per core): k/v up-projection
    from gathered kv latents; causal flash attention (k-major scores, exp on
    ScalarE, softmax denominator accumulated on the Pool engine, one
    ones-matmul per q-tile, reciprocal + K=1 broadcast matmul for the divide).
    Inner loop software-pipelined: V-matmul lags the score matmuls by two
    chunks so the PE never waits on the exp.
  output: two AllToAlls (even heads overlap the odd-head attention; odd heads
    overlap the even half of the token-parallel output projection). w_o and
    the o-receives prefetch during attention.

bf16 matmuls, fp32 PSUM accumulation + softmax statistics, fp32 output.
"""
import sys
from contextlib import ExitStack

sys.path.insert(0, "/opt/trn_rl_repo")

import numpy as np
import ml_dtypes

import concourse.bacc as bacc
import concourse.mybir as mybir
import concourse.tile as tile
from concourse.bass_utils import run_bass_kernel_spmd

# ---- problem sizes (hardcoded per spec) ----
HID = 2048; H = 16; QLR = 1536; KVLR = 512
DN = 128; DR = 64; DV = 128; DQ = DN + DR
B = 2; S = 2048
THETA = 10000.0; EPS = 1e-6

NCORES = 8
T = B * S              # 4096 flattened tokens
TPC = T // NCORES      # 512 tokens per core
HPC = H // NCORES      # 2 heads per core
P = 128
NHID = HID // P        # 16
NQLR = QLR // P        # 12
CKW = KVLR + DR        # 576
QT_PER_B = S // 512    # 4 q-tiles of 512 per (b,h) unit
WKK = HPC * DN         # 256

BF16 = mybir.dt.bfloat16
F32 = mybir.dt.float32
AF = mybir.ActivationFunctionType

_NC_CACHE = None


def _rope_dual(nc, pool, out_bf16, ps, cos_sb, sin_sb, tag, scale=None):
    """RoPE on a [128, W] psum holding two 64-row head groups; writes bf16.

    If ``scale`` is given (a [128, W] broadcast of a per-token scalar), the
    output is additionally multiplied by it (folded q rmsnorm)."""
    W = 512
    HDR = DR // 2
    rot = pool.tile([P, W], F32, tag=f"{tag}rot", name=f"{tag}rot")
    for g in range(2):
        o = g * DR
        nc.scalar.mul(rot[o:o + HDR, :], ps[o + HDR:o + DR, :], -1.0)
        nc.scalar.copy(rot[o + HDR:o + DR, :], ps[o:o + HDR, :])
    t1 = pool.tile([P, W], F32, tag=f"{tag}t1", name=f"{tag}t1")
    nc.vector.tensor_mul(t1[:], ps[:], cos_sb[:])
    nc.vector.tensor_mul(rot[:], rot[:], sin_sb[:])
    if scale is None:
        nc.vector.tensor_add(out_bf16[:], t1[:], rot[:])
    else:
        tsum = pool.tile([P, W], F32, tag=f"{tag}sum", name=f"{tag}sum")
        nc.vector.tensor_add(tsum[:], t1[:], rot[:])
        nc.vector.tensor_mul(out_bf16[:], tsum[:], scale[:])


def build_nc():
    nc = bacc.Bacc(None, target_bir_lowering=False, debug=False, num_devices=NCORES)

    # ---- per-core external inputs ----
    hidT = nc.dram_tensor("hidT", [HID, TPC], BF16, kind="ExternalInput")
    wqaT = nc.dram_tensor("wqaT", [HID, QLR], BF16, kind="ExternalInput")
    wkvaT = nc.dram_tensor("wkvaT", [HID, CKW], BF16, kind="ExternalInput")
    wqbT = nc.dram_tensor("wqbT", [QLR, H * DQ], BF16, kind="ExternalInput")
    wkvbkT = nc.dram_tensor("wkvbkT", [KVLR, HPC * DN], BF16, kind="ExternalInput")
    wkvbvT = nc.dram_tensor("wkvbvT", [KVLR, HPC * DV], BF16, kind="ExternalInput")
    woT = nc.dram_tensor("woT", [H * DV, HID], BF16, kind="ExternalInput")
    cosd = nc.dram_tensor("cosd", [P, TPC], BF16, kind="ExternalInput")
    sind = nc.dram_tensor("sind", [P, TPC], BF16, kind="ExternalInput")
    masks = nc.dram_tensor("masks", [P, 4 * 512], BF16, kind="ExternalInput")
    outT = nc.dram_tensor("outT", [HID, TPC], F32, kind="ExternalOutput")

    RG = [list(range(NCORES))]

    with tile.TileContext(nc) as tc:
        with tc.tile_pool(name="dram", bufs=1, space="DRAM") as dram, \
             tc.tile_pool(name="const", bufs=1) as const, \
             tc.tile_pool(name="pwk", bufs=1) as pwk:
            latkv_in = dram.tile([CKW, TPC], BF16)
            latkv_all = dram.tile([NCORES * CKW, TPC], BF16, addr_space="Shared")
            qa_in = dram.tile([NCORES * 2 * P, TPC], BF16)   # [pe2|h0n] per pair
            qa_out = dram.tile([NCORES * 2 * P, TPC], BF16)
            qb_in = dram.tile([NCORES * P, TPC], BF16)       # h1n per pair
            qb_out = dram.tile([NCORES * P, TPC], BF16)
            oa_in = dram.tile([NCORES * DV, TPC], BF16)      # even heads out
            oa_out = dram.tile([NCORES * DV, TPC], BF16)
            ob_in = dram.tile([NCORES * DV, TPC], BF16)      # odd heads out
            ob_out = dram.tile([NCORES * DV, TPC], BF16)

            ones_col = const.tile([P, 1], BF16)
            nc.vector.memset(ones_col[:], 1.0)
            ones_row = const.tile([1, P], F32)
            nc.vector.memset(ones_row[:], 1.0)
            ones_row_b = const.tile([1, P], BF16)
            nc.vector.memset(ones_row_b[:], 1.0)
            eps_t = const.tile([1, 1], F32)
            nc.vector.memset(eps_t[:], EPS)
            cos_sb = const.tile([P, TPC], BF16)
            sin_sb = const.tile([P, TPC], BF16)
            mask_sb = const.tile([P, 4 * 512], BF16)
            wkk_sb = pwk.tile([P, 4 * WKK], BF16)
            wkv_sb = pwk.tile([P, 4 * WKK], BF16)
            ckvj_bufs = [pwk.tile([P, 4 * TPC], BF16, tag=f"ckvj{b}",
                                  name=f"ckvj{b}") for b in range(NCORES)]
            kpe_t = [pwk.tile([P, S], BF16, tag=f"kp{bb}", name=f"kp{bb}")
                     for bb in range(2)]

            # ====== phase-1 long-lived pool (wqb prefetch + normed cq) ======
            p1q_stack = ExitStack()
            p1q = p1q_stack.enter_context(tc.tile_pool(name="p1q", bufs=1))
            ps1_stack = ExitStack()
            ps1 = ps1_stack.enter_context(tc.tile_pool(name="ps1", bufs=3, space="PSUM"))
            ps1b_stack = ExitStack()
            ps1s = ps1b_stack.enter_context(tc.tile_pool(name="ps1s", bufs=1, space="PSUM"))
            ps1b = ps1b_stack.enter_context(tc.tile_pool(name="ps1b", bufs=2, space="PSUM"))

            WQBH = (H * DQ) // 2  # 1536 output dims per load half
            wqb_ch = [p1q.tile([P, WQBH], BF16, tag=f"wqb{kc}", name=f"wqb{kc}")
                      for kc in range(NQLR)]
            cqn_sb = p1q.tile([P, NQLR * TPC], BF16)

            # ============ Phase 1a/1b: token-parallel down-proj + norms ============
            p1ab_stack = ExitStack()
            p1a = p1ab_stack.enter_context(tc.tile_pool(name="p1a", bufs=1))
            p1t = p1ab_stack.enter_context(tc.tile_pool(name="p1t", bufs=2))
            p1r = p1ab_stack.enter_context(tc.tile_pool(name="p1r", bufs=1))
            p1n = p1ab_stack.enter_context(tc.tile_pool(name="p1n", bufs=1))

            hid_ch = [p1a.tile([P, TPC], BF16, tag=f"hid{kc}", name=f"hid{kc}")
                      for kc in range(NHID)]
            wkva_ch = [p1a.tile([P, CKW], BF16, tag=f"wkva{kc}", name=f"wkva{kc}")
                       for kc in range(NHID)]
            wqa_ch = [p1a.tile([P, QLR], BF16, tag=f"wqa{kc}", name=f"wqa{kc}")
                      for kc in range(NHID)]
            ckv_bf = p1a.tile([P, 4 * TPC], BF16)

            # DMA issue order = consumption order: hid+wkva feed ckv first,
            # then rope tables, then wqa (cq), then the big wqb prefetch.
            for kc in range(NHID):
                nc.sync.dma_start(hid_ch[kc][:], hidT.ap()[kc * P:(kc + 1) * P, :])
                nc.sync.dma_start(wkva_ch[kc][:], wkvaT.ap()[kc * P:(kc + 1) * P, :])
            nc.sync.dma_start(cos_sb[:], cosd.ap()[:])
            nc.sync.dma_start(sin_sb[:], sind.ap()[:])
            for kc in range(NHID):
                nc.sync.dma_start(wqa_ch[kc][:], wqaT.ap()[kc * P:(kc + 1) * P, :])
            for kc in range(NQLR):
                nc.sync.dma_start(wqb_ch[kc][:],
                                  wqbT.ap()[kc * P:(kc + 1) * P, 0:WQBH])
            for kc in range(4):
                nc.sync.dma_start(wkk_sb[:, kc * WKK:(kc + 1) * WKK],
                                  wkvbkT.ap()[kc * P:(kc + 1) * P, :])
                nc.sync.dma_start(wkv_sb[:, kc * WKK:(kc + 1) * WKK],
                                  wkvbvT.ap()[kc * P:(kc + 1) * P, :])
            nc.sync.dma_start(mask_sb[:], masks.ap()[:])

            # --- ckv joint (d-major): 4 normed blocks + k_pe block.
            # Sum-of-squares matmuls lag one block so the PE never waits.
            ssq_kv = ps1s.tile([1, TPC], F32, tag="ssqkv")
            sq_kv = []
            for m in range(4):
                ps = ps1.tile([P, TPC], F32, tag="proj")
                for kc in range(NHID):
                    nc.tensor.matmul(ps[:], wkva_ch[kc][:, m * P:(m + 1) * P],
                                     hid_ch[kc][:],
                                     start=(kc == 0), stop=(kc == NHID - 1))
                nc.scalar.copy(ckv_bf[:, m * TPC:(m + 1) * TPC], ps[:])
                sq = p1t.tile([P, TPC], BF16, tag="sq")
                nc.vector.tensor_mul(sq[:], ckv_bf[:, m * TPC:(m + 1) * TPC],
                                     ckv_bf[:, m * TPC:(m + 1) * TPC])
                sq_kv.append(sq)
                if m >= 1:
                    nc.tensor.matmul(ssq_kv[:], ones_col[:], sq_kv[m - 1][:],
                                     start=(m == 1), stop=False,
                                     skip_group_check=True)

            # k_pe block [64, TPC] (shared across heads)
            ps_pe = ps1.tile([DR, TPC], F32, tag="proj")
            for kc in range(NHID):
                nc.tensor.matmul(ps_pe[:], wkva_ch[kc][:, KVLR:CKW],
                                 hid_ch[kc][:],
                                 start=(kc == 0), stop=(kc == NHID - 1))
            nc.tensor.matmul(ssq_kv[:], ones_col[:], sq_kv[3][:],
                             start=False, stop=True, skip_group_check=True)

            # k_pe rope -> latkv_in tail
            HDR = DR // 2
            rot = p1r.tile([DR, TPC], F32, tag="rot")
            nc.scalar.mul(rot[0:HDR, :], ps_pe[HDR:DR, :], -1.0)
            nc.scalar.copy(rot[HDR:DR, :], ps_pe[0:HDR, :])
            t1 = p1r.tile([DR, TPC], F32, tag="t1")
            nc.vector.tensor_mul(t1[:], ps_pe[:], cos_sb[0:DR, :])
            nc.vector.tensor_mul(rot[:], rot[:], sin_sb[0:DR, :])
            pe_out = p1r.tile([DR, TPC], BF16, tag="peo")
            nc.vector.tensor_add(pe_out[:], t1[:], rot[:])
            nc.gpsimd.dma_start(latkv_in[KVLR:CKW, :], pe_out[:])

            # kv norm scalars (Act + DVE) while the PE moves on to cq
            kv_norm = p1n.tile([1, TPC], F32, tag="nrm")
            nc.scalar.activation(kv_norm[:], ssq_kv[:], AF.Sqrt, bias=eps_t[:],
                                 scale=1.0 / KVLR)
            rn_kv = p1n.tile([1, TPC], F32, tag="rn")
            nc.vector.reciprocal_approx_fast(rn_kv[:], kv_norm[:])

            # --- cq blocks land unnormalized in cqn_sb; normalized in place
            ssq_q = ps1s.tile([1, TPC], F32, tag="ssqq")
            sq_q = []

            def cq_block(m):
                ps = ps1.tile([P, TPC], F32, tag="proj", name="ps")
                for kc in range(NHID):
                    nc.tensor.matmul(ps[:], wqa_ch[kc][:, m * P:(m + 1) * P],
                                     hid_ch[kc][:],
                                     start=(kc == 0), stop=(kc == NHID - 1))
                nc.scalar.copy(cqn_sb[:, m * TPC:(m + 1) * TPC], ps[:])
                sq = p1t.tile([P, TPC], BF16, tag="sq", name="sq")
                nc.vector.tensor_mul(sq[:], cqn_sb[:, m * TPC:(m + 1) * TPC],
                                     cqn_sb[:, m * TPC:(m + 1) * TPC])
                sq_q.append(sq)

            cq_block(0)
            cq_block(1)
            cq_block(2)

            # kv latents: normalize + ship + AllGather
            bkv = ps1b.tile([P, TPC], F32, tag="bc")
            nc.tensor.matmul(bkv[:], ones_row[:], rn_kv[:], start=True, stop=True)
            for m in range(4):
                lat_o = p1t.tile([P, TPC], BF16, tag="sq")
                nc.vector.tensor_mul(lat_o[:], ckv_bf[:, m * TPC:(m + 1) * TPC], bkv[:])
                nc.gpsimd.dma_start(latkv_in[m * P:(m + 1) * P, :], lat_o[:])
            nc.gpsimd.collective_compute(
                "AllGather", mybir.AluOpType.bypass, replica_groups=RG,
                ins=[latkv_in.opt()], outs=[latkv_all.opt()])

            # --- cq blocks 3..11, ssq lagging one block ---
            nc.tensor.matmul(ssq_q[:], ones_col[:], sq_q[0][:],
                             start=True, stop=False, skip_group_check=True)
            nc.tensor.matmul(ssq_q[:], ones_col[:], sq_q[1][:],
                             start=False, stop=False, skip_group_check=True)
            for m in range(3, NQLR):
                cq_block(m)
                nc.tensor.matmul(ssq_q[:], ones_col[:], sq_q[m - 1][:],
                                 start=False, stop=False,
                                 skip_group_check=True)
            nc.tensor.matmul(ssq_q[:], ones_col[:], sq_q[NQLR - 1][:],
                             start=False, stop=True, skip_group_check=True)

            sq_norm = p1n.tile([1, TPC], F32, tag="nrm")
            nc.scalar.activation(sq_norm[:], ssq_q[:], AF.Sqrt, bias=eps_t[:],
                                 scale=1.0 / QLR)
            rn_q = p1n.tile([1, TPC], F32, tag="rn")
            nc.vector.reciprocal_approx_fast(rn_q[:], sq_norm[:])
            bq_ps = ps1b.tile([P, TPC], F32, tag="bc")
            nc.tensor.matmul(bq_ps[:], ones_row[:], rn_q[:], start=True, stop=True)
            bq = p1q.tile([P, TPC], F32)
            nc.scalar.copy(bq[:], bq_ps[:])

            ps1b_stack.close()
            p1ab_stack.close()   # frees hid/wkva/wqa/ckv sbuf

            # second half of wqb: separate pool reusing the freed phase-1a
            # space, loaded immediately (no reload seam mid q-up)
            p1wb_stack = ExitStack()
            p1wb = p1wb_stack.enter_context(tc.tile_pool(name="p1wb", bufs=1))
            wqb_chB = [p1wb.tile([P, WQBH], BF16, tag=f"wqbB{kc}",
                                 name=f"wqbB{kc}") for kc in range(NQLR)]
            for kc in range(NQLR):
                nc.sync.dma_start(wqb_chB[kc][:],
                                  wqbT.ap()[kc * P:(kc + 1) * P, WQBH:2 * WQBH])

            # gathered latents + shared rope keys prefetch into SBUF the
            # moment the AllGather lands (before the q AllToAlls hold the
            # DMA rings)
            for j in range(NCORES):
                basek = j * CKW
                for r in range(4):
                    nc.gpsimd.dma_start(
                        ckvj_bufs[j][:, r * TPC:(r + 1) * TPC],
                        latkv_all[basek + r * P: basek + (r + 1) * P, :])
                nc.gpsimd.dma_start(
                    kpe_t[j // 4][0:DR, (j % 4) * TPC:(j % 4 + 1) * TPC],
                    latkv_all[basek + KVLR: basek + CKW, :])
                nc.gpsimd.dma_start(
                    kpe_t[j // 4][DR:P, (j % 4) * TPC:(j % 4 + 1) * TPC],
                    latkv_all[basek + KVLR: basek + CKW, :])

            # ============ Phase 1c: q up-projection for ALL heads ============
            with tc.tile_pool(name="p1qt", bufs=8) as p1qt, \
                 tc.tile_pool(name="p1qr", bufs=1) as p1qr:
                wqb_cur = wqb_ch

                def qup_block(mb):
                    col = (mb % 12) * P
                    ps = ps1.tile([P, TPC], F32, tag="proj", name="ps")
                    for kc in range(NQLR):
                        nc.tensor.matmul(ps[:], wqb_cur[kc][:, col:col + P],
                                         cqn_sb[:, kc * TPC:(kc + 1) * TPC],
                                         start=(kc == 0), stop=(kc == NQLR - 1))
                    qo = p1qt.tile([P, TPC], BF16, tag="qo", name="qo")
                    if mb < 16 and mb % 2 == 0:  # pe2 block -> rope + rmsnorm
                        _rope_dual(nc, p1qr, qo, ps, cos_sb, sin_sb, "q",
                                   scale=bq)
                    else:
                        nc.vector.tensor_mul(qo[:], ps[:], bq[:])
                    if mb < 16:
                        nc.scalar.dma_start(qa_in[mb * P:(mb + 1) * P, :], qo[:])
                    else:
                        mo = mb - 16
                        nc.scalar.dma_start(qb_in[mo * P:(mo + 1) * P, :], qo[:])

                for mb in range(12):
                    qup_block(mb)
                wqb_cur = wqb_chB
                for mb in range(12, 16):
                    qup_block(mb)
                nc.gpsimd.collective_compute(
                    "AllToAll", mybir.AluOpType.bypass, replica_groups=RG,
                    ins=[qa_in.opt()], outs=[qa_out.opt()])
                for mb in range(16, 24):
                    qup_block(mb)
                nc.gpsimd.collective_compute(
                    "AllToAll", mybir.AluOpType.bypass, replica_groups=RG,
                    ins=[qb_in.opt()], outs=[qb_out.opt()])
            p1wb_stack.close()
            ps1_stack.close()
            p1q_stack.close()    # frees wqb + cqn

            # ====== phase-3 staging pool (lives to the end) ======
            p3w_stack = ExitStack()
            p3w = p3w_stack.enter_context(tc.tile_pool(name="p3w", bufs=1))

            # attention-lifetime kv tiles (split per head / per batch so the
            # first attention unit only depends on its own producers)
            atta_stack = ExitStack()
            att_a = atta_stack.enter_context(tc.tile_pool(name="att_a", bufs=1))
            knope_t = [[att_a.tile([P, S], BF16, tag=f"kn{hl}{bb}",
                                   name=f"kn{hl}{bb}") for bb in range(2)]
                       for hl in range(2)]
            v_t = [att_a.tile([P, 16 * WKK], BF16, tag=f"v{bb}", name=f"v{bb}")
                   for bb in range(2)]

            # ============ Phase 2: k/v up-projection (b0 then b1) ============
            with tc.tile_pool(name="ps2", bufs=4, space="PSUM") as ps2:
                for bbv in range(2):
                    for jj in range(4):
                        j = bbv * 4 + jj
                        ckv_j = ckvj_bufs[j]
                        for m in range(HPC):
                            ps = ps2.tile([P, TPC], F32, tag="proj")
                            for kc in range(4):
                                nc.tensor.matmul(
                                    ps[:], wkk_sb[:, kc * WKK + m * P: kc * WKK + (m + 1) * P],
                                    ckv_j[:, kc * TPC:(kc + 1) * TPC],
                                    start=(kc == 0), stop=(kc == 3))
                            nc.scalar.copy(
                                knope_t[m][bbv][:, jj * TPC:(jj + 1) * TPC], ps[:])
                        for tb in range(4):
                            ps = ps2.tile([P, WKK], F32, tag="vproj")
                            for kc in range(4):
                                nc.tensor.matmul(
                                    ps[:],
                                    ckv_j[:, kc * TPC + tb * P: kc * TPC + (tb + 1) * P],
                                    wkv_sb[:, kc * WKK:(kc + 1) * WKK],
                                    start=(kc == 0), stop=(kc == 3))
                            jb = jj * 4 + tb
                            nc.scalar.copy(
                                v_t[bbv][:, jb * WKK:(jb + 1) * WKK], ps[:])

            # w_o even-half prefetch (fires once the wqb space is released)
            woe_sb = p3w.tile([P, NCORES * HID], BF16)
            for i in range(NCORES):
                nc.sync.dma_start(woe_sb[:, i * HID:(i + 1) * HID],
                                  woT.ap()[(2 * i) * P:(2 * i + 1) * P, :])

            # q receives (fire when the AllToAlls deliver); split per head
            # and per batch for precise attention dependencies
            qpe_stack = ExitStack()
            attq = qpe_stack.enter_context(tc.tile_pool(name="attq", bufs=1))
            qnope_t = [[attq.tile([P, S], BF16, tag=f"qn{hl}{bb}",
                                  name=f"qn{hl}{bb}") for bb in range(2)]
                       for hl in range(2)]
            qpe_t = [attq.tile([P, S], BF16, tag=f"qp{bb}", name=f"qp{bb}")
                     for bb in range(2)]
            for i in range(NCORES):
                bb, col = i // 4, (i % 4) * TPC
                nc.gpsimd.dma_start(qpe_t[bb][:, col:col + TPC],
                                    qa_out[i * 2 * P: i * 2 * P + P, :])
                nc.gpsimd.dma_start(qnope_t[0][bb][:, col:col + TPC],
                                    qa_out[i * 2 * P + P: (i + 1) * 2 * P, :])

            # ============ attention (4 causal units, hl-major) ============
            with tc.tile_pool(name="att_t", bufs=2) as att_t, \
                 tc.tile_pool(name="att_e", bufs=2) as att_e, \
                 tc.tile_pool(name="att_x", bufs=3) as att_x, \
                 tc.tile_pool(name="ps_s", bufs=4, space="PSUM") as ps_s_pool, \
                 tc.tile_pool(name="ps_o", bufs=2, space="PSUM") as ps_o_pool, \
                 tc.tile_pool(name="ps_d", bufs=1, space="PSUM") as ps_d_pool:

                for u in range(4):  # hl-major: (hl, bb)
                    hl, bb = u // 2, u % 2
                    kn = knope_t[hl][bb]
                    qn = qnope_t[hl][bb]
                    chunks = [(qt, kc) for qt in range(QT_PER_B)
                              for kc in range(4 * (qt + 1))]
                    exs = {}
                    psos = {}
                    Es = {}
                    fin_pending = []

                    def emit_v(idx, chunks=chunks, exs=exs, psos=psos, Es=Es,
                               hl=hl, bb=bb):
                        qt, kc = chunks[idx]
                        nkc = 4 * (qt + 1)
                        nc.tensor.matmul(
                            psos[qt][:],
                            v_t[bb][:, kc * WKK + hl * DV: kc * WKK + (hl + 1) * DV],
                            exs.pop(idx)[:],
                            start=(kc == 0), stop=(kc == nkc - 1),
                            skip_group_check=True)
                        if kc == nkc - 1:
                            # softmax denominator: one M=1 matmul per q-tile;
                            # the broadcast + divide are deferred two chunks so
                            # the PE never waits on the reciprocal
                            ps_den = ps_d_pool.tile([1, 512], F32, tag="psd",
                                                    name="ps_den")
                            nc.tensor.matmul(ps_den[:], ones_col[:],
                                             Es.pop(qt)[:],
                                             start=True, stop=True,
                                             skip_group_check=True)
                            rec_f = att_t.tile([1, 512], F32, tag="rcf",
                                               name="rec_f")
                            nc.vector.reciprocal_approx_fast(rec_f[:], ps_den[:])
                            rec_b = att_t.tile([1, 512], BF16, tag="rcb",
                                               name="rec_b")
                            nc.vector.tensor_copy(rec_b[:], rec_f[:])
                            fin_pending.append((qt, rec_b))

                    def emit_fin():
                        qt, rec_b = fin_pending.pop(0)
                        bc = ps_s_pool.tile([P, 512], F32, tag="pss", name="bc")
                        nc.tensor.matmul(bc[:], ones_row_b[:], rec_b[:],
                                         start=True, stop=True,
                                         skip_group_check=True)
                        bc_sb = att_t.tile([P, 512], BF16, tag="bcs",
                                           name="bc_sb")
                        nc.scalar.copy(bc_sb[:], bc[:])
                        on = att_t.tile([P, 512], BF16, tag="on", name="on")
                        nc.vector.tensor_mul(on[:], psos.pop(qt)[:], bc_sb[:])
                        blk = bb * QT_PER_B + qt
                        tgt = oa_in if hl == 0 else ob_in
                        nc.gpsimd.dma_start(tgt[blk * DV:(blk + 1) * DV, :], on[:])

                    for idx, (qt, kc) in enumerate(chunks):
                        if kc == 0:
                            psos[qt] = ps_o_pool.tile([P, 512], F32, tag="pso",
                                                      name="pso")
                            Es[qt] = att_e.tile([P, 512], BF16, tag="E", name="E")
                        qoff = qt * 512
                        koff = kc * P
                        ps_sc = ps_s_pool.tile([P, 512], F32, tag="pss")
                        nc.tensor.matmul(
                            ps_sc[:], kn[:, koff: koff + P],
                            qn[:, qoff: qoff + 512],
                            start=True, stop=False)
                        nc.tensor.matmul(
                            ps_sc[:], kpe_t[bb][hl * DR: hl * DR + DR, koff: koff + P],
                            qpe_t[bb][hl * DR: hl * DR + DR, qoff: qoff + 512],
                            start=False, stop=True)
                        ex = att_x.tile([P, 512], BF16, tag="ex")
                        nc.scalar.activation(ex[:], ps_sc[:], AF.Exp)
                        if kc >= 4 * qt:
                            mi = kc - 4 * qt
                            nc.vector.tensor_mul(ex[:], ex[:],
                                                 mask_sb[:, mi * 512:(mi + 1) * 512])
                        if kc == 0:
                            nc.vector.tensor_copy(Es[qt][:], ex[:])
                        else:
                            nc.vector.tensor_add(Es[qt][:], Es[qt][:], ex[:])
                        exs[idx] = ex
                        if idx >= 2:
                            emit_v(idx - 2)
                            if kc >= 2 and fin_pending:
                                emit_fin()
                    emit_v(len(chunks) - 2)
                    emit_v(len(chunks) - 1)
                    while fin_pending:
                        emit_fin()

                    if u == 0:
                        # odd-head q receives fire once the qb AllToAll lands
                        for i in range(NCORES):
                            bbq, col = i // 4, (i % 4) * TPC
                            nc.gpsimd.dma_start(qnope_t[1][bbq][:, col:col + TPC],
                                                qb_out[i * P:(i + 1) * P, :])
                    if u == 1:  # even heads complete -> overlap with odd attention
                        nc.gpsimd.collective_compute(
                            "AllToAll", mybir.AluOpType.bypass, replica_groups=RG,
                            ins=[oa_in.opt()], outs=[oa_out.opt()])
                        # even-head receives + odd-half w_o prefetch during
                        # the odd-head attention
                        oe_sb = p3w.tile([P, NCORES * TPC], BF16)
                        for i in range(NCORES):
                            nc.sync.dma_start(oe_sb[:, i * TPC:(i + 1) * TPC],
                                              oa_out[i * P:(i + 1) * P, :])
                        woo_sb = p3w.tile([P, NCORES * HID], BF16)
                        for i in range(NCORES):
                            nc.sync.dma_start(woo_sb[:, i * HID:(i + 1) * HID],
                                              woT.ap()[(2 * i + 1) * P:(2 * i + 2) * P, :])

                nc.gpsimd.collective_compute(
                    "AllToAll", mybir.AluOpType.bypass, replica_groups=RG,
                    ins=[ob_in.opt()], outs=[ob_out.opt()])
                oo_sb = p3w.tile([P, NCORES * TPC], BF16)
                for i in range(NCORES):
                    nc.sync.dma_start(oo_sb[:, i * TPC:(i + 1) * TPC],
                                      ob_out[i * P:(i + 1) * P, :])

            qpe_stack.close()
            atta_stack.close()

            # ============ Phase 3: two passes (pass 1 overlaps the ob AllToAll) ============
            with tc.tile_pool(name="p3p", bufs=1) as p3p, \
                 tc.tile_pool(name="p3t", bufs=3) as p3t, \
                 tc.tile_pool(name="ps3", bufs=4, space="PSUM") as ps3:
                part_sb = p3p.tile([P, NHID * TPC], F32)
                for m in range(NHID):
                    ps = ps3.tile([P, TPC], F32, tag="proj")
                    for i in range(NCORES):
                        nc.tensor.matmul(
                            ps[:], woe_sb[:, i * HID + m * P: i * HID + (m + 1) * P],
                            oe_sb[:, i * TPC:(i + 1) * TPC],
                            start=(i == 0), stop=(i == NCORES - 1))
                    nc.scalar.copy(part_sb[:, m * TPC:(m + 1) * TPC], ps[:])
                for m in range(NHID):
                    ps = ps3.tile([P, TPC], F32, tag="proj")
                    for i in range(NCORES):
                        nc.tensor.matmul(
                            ps[:], woo_sb[:, i * HID + m * P: i * HID + (m + 1) * P],
                            oo_sb[:, i * TPC:(i + 1) * TPC],
                            start=(i == 0), stop=(i == NCORES - 1))
                    ot = p3t.tile([P, TPC], F32, tag="ot")
                    nc.vector.tensor_add(ot[:], ps[:], part_sb[:, m * TPC:(m + 1) * TPC])
                    nc.sync.dma_start(outT.ap()[m * P:(m + 1) * P, :], ot[:])
            p3w_stack.close()
    nc.finalize()
    return nc


def _bf16(x):
    return np.ascontiguousarray(x.astype(ml_dtypes.bfloat16))


def _rope_tables():
    inv_freq = 1.0 / (THETA ** (np.arange(0, DR, 2, dtype=np.float64) / DR))
    t = np.arange(S, dtype=np.float64)
    freqs = np.outer(t, inv_freq)
    emb = np.concatenate((freqs, freqs), axis=-1)
    return np.cos(emb).astype(np.float32), np.sin(emb).astype(np.float32)


def prepare_inputs(hidden_states, w_qa, q_a_ln_w, w_qb, w_kva, kv_a_ln_w, w_kvb, w_o):
    hidden_states = np.asarray(hidden_states, dtype=np.float32)
    w_qa = np.asarray(w_qa, dtype=np.float32)
    q_a_ln_w = np.asarray(q_a_ln_w, dtype=np.float32)
    w_qb = np.asarray(w_qb, dtype=np.float32)
    w_kva = np.asarray(w_kva, dtype=np.float32)
    kv_a_ln_w = np.asarray(kv_a_ln_w, dtype=np.float32)
    w_kvb = np.asarray(w_kvb, dtype=np.float32)
    w_o = np.asarray(w_o, dtype=np.float32)

    flat = hidden_states.reshape(T, HID)
    cos, sin = _rope_tables()          # [S, DR]
    scale = DQ ** -0.5

    pos = np.arange(T) % S
    cos_d = cos[pos].T                 # [DR, T]
    sin_d = sin[pos].T

    kp = np.arange(P)[:, None]
    qf = np.arange(512)[None, :]
    masks = _bf16(np.concatenate(
        [(qf >= kp + P * p).astype(np.float32) for p in range(4)], axis=1))

    w_qb_eff = (w_qb * q_a_ln_w[None, :]) * scale       # [H*DQ, QLR]
    w_kvb_eff = w_kvb * kv_a_ln_w[None, :]              # [H*(DN+DV), KVLR]

    # w_qb rows permuted: block A = per pair j [h0 pe | h1 pe | h0 nope],
    # block B = per pair j [h1 nope]
    rows = []
    for j in range(NCORES):
        h0, h1 = 2 * j, 2 * j + 1
        rows.append(w_qb_eff[h0 * DQ + DN: h0 * DQ + DQ])   # h0 pe (64)
        rows.append(w_qb_eff[h1 * DQ + DN: h1 * DQ + DQ])   # h1 pe (64)
        rows.append(w_qb_eff[h0 * DQ: h0 * DQ + DN])        # h0 nope (128)
    for j in range(NCORES):
        h1 = 2 * j + 1
        rows.append(w_qb_eff[h1 * DQ: h1 * DQ + DN])        # h1 nope (128)
    wqbT_full = _bf16(np.concatenate(rows, axis=0).T)       # [QLR, 3072]

    wqaT = _bf16(w_qa.T)
    wkvaT = _bf16(w_kva.T)
    woT = _bf16(w_o.T)

    in_maps = []
    for c in range(NCORES):
        heads = [HPC * c + h for h in range(HPC)]
        krows = [w_kvb_eff[h * (DN + DV): h * (DN + DV) + DN] for h in heads]
        wkvbkT_c = _bf16(np.concatenate(krows, axis=0).T)
        vrows = [w_kvb_eff[h * (DN + DV) + DN: (h + 1) * (DN + DV)] for h in heads]
        wkvbvT_c = _bf16(np.concatenate(vrows, axis=0).T)

        tok0 = c * TPC
        cosl = cos_d[:, tok0:tok0 + TPC]
        sinl = sin_d[:, tok0:tok0 + TPC]
        in_maps.append({
            "hidT": _bf16(flat[tok0:tok0 + TPC].T),
            "wqaT": wqaT, "wkvaT": wkvaT,
            "wqbT": wqbT_full, "wkvbkT": wkvbkT_c, "wkvbvT": wkvbvT_c,
            "woT": woT,
            "cosd": _bf16(np.concatenate([cosl, cosl], axis=0)),
            "sind": _bf16(np.concatenate([sinl, sinl], axis=0)),
            "masks": masks,
        })
    return in_maps


def kernel(hidden_states, w_qa, q_a_ln_w, w_qb, w_kva, kv_a_ln_w, w_kvb, w_o,
           _trace=False):
    global _NC_CACHE
    if _NC_CACHE is None:
        _NC_CACHE = build_nc()
    nc = _NC_CACHE
    in_maps = prepare_inputs(hidden_states, w_qa, q_a_ln_w, w_qb, w_kva,
                             kv_a_ln_w, w_kvb, w_o)
    res = run_bass_kernel_spmd(nc, in_maps, core_ids=list(range(NCORES)),
                               trace=_trace)
    out = np.empty((T, HID), dtype=np.float32)
    for c in range(NCORES):
        out[c * TPC:(c + 1) * TPC] = res.results[c]["outT"].T
    if _trace:
        kernel._last_result = res
    return out.reshape(B, S, HID)


# revision 19
# speedup vs baseline: 1.1123x; 1.1123x over previous
"""MLA (DeepSeek-style) attention block on 8 Trainium2 NeuronCores.

Sharding:
  phase 1 (token-parallel, 8 x 512 tokens): LoRA-A down-projections + rmsnorm
    + k_pe rope; small AllGather of the kv latents (576 dims); q up-projection
    for ALL 16 heads on the token side + rope, shipped to head shards via two
    shard-aligned AllToAlls (pe+even-nope first, odd-nope second).
  phase 2 (head-parallel, 2 heads x 2 batches per core): k/v up-projection
    from gathered kv latents; causal flash attention (k-major scores, exp on
    ScalarE, softmax denominator accumulated on the Pool engine, one
    ones-matmul per q-tile, reciprocal + K=1 broadcast matmul for the divide).
    Inner loop software-pipelined: V-matmul lags the score matmuls by two
    chunks so the PE never waits on the exp.
  output: two AllToAlls (even heads overlap the odd-head attention; odd heads
    overlap the even half of the token-parallel output projection). w_o and
    the o-receives prefetch during attention.

bf16 matmuls, fp32 PSUM accumulation + softmax statistics, fp32 output.
"""
import sys
from contextlib import ExitStack

sys.path.insert(0, "/opt/trn_rl_repo")

import numpy as np
import ml_dtypes

import concourse.bacc as bacc
import concourse.mybir as mybir
import concourse.tile as tile
from concourse.bass_utils import run_bass_kernel_spmd

# ---- problem sizes (hardcoded per spec) ----
HID = 2048; H = 16; QLR = 1536; KVLR = 512
DN = 128; DR = 64; DV = 128; DQ = DN + DR
B = 2; S = 2048
THETA = 10000.0; EPS = 1e-6

NCORES = 8
T = B * S              # 4096 flattened tokens
TPC = T // NCORES      # 512 tokens per core
HPC = H // NCORES      # 2 heads per core
P = 128
NHID = HID // P        # 16
NQLR = QLR // P        # 12
CKW = KVLR + DR        # 576
QT_PER_B = S // 512    # 4 q-tiles of 512 per (b,h) unit
WKK = HPC * DN         # 256

BF16 = mybir.dt.bfloat16
F32 = mybir.dt.float32
AF = mybir.ActivationFunctionType

_NC_CACHE = None


def _rope_dual(nc, pool, out_bf16, ps, cos_sb, sin_sb, tag, scale=None):
    """RoPE on a [128, W] psum holding two 64-row head groups; writes bf16.

    If ``scale`` is given (a [128, W] broadcast of a per-token scalar), the
    output is additionally multiplied by it (folded q rmsnorm)."""
    W = 512
    HDR = DR // 2
    rot = pool.tile([P, W], F32, tag=f"{tag}rot", name=f"{tag}rot")
    for g in range(2):
        o = g * DR
        nc.scalar.mul(rot[o:o + HDR, :], ps[o + HDR:o + DR, :], -1.0)
        nc.scalar.copy(rot[o + HDR:o + DR, :], ps[o:o + HDR, :])
    t1 = pool.tile([P, W], F32, tag=f"{tag}t1", name=f"{tag}t1")
    nc.vector.tensor_mul(t1[:], ps[:], cos_sb[:])
    nc.vector.tensor_mul(rot[:], rot[:], sin_sb[:])
    if scale is None:
        nc.vector.tensor_add(out_bf16[:], t1[:], rot[:])
    else:
        tsum = pool.tile([P, W], F32, tag=f"{tag}sum", name=f"{tag}sum")
        nc.vector.tensor_add(tsum[:], t1[:], rot[:])
        nc.vector.tensor_mul(out_bf16[:], tsum[:], scale[:])


def build_nc():
    nc = bacc.Bacc(None, target_bir_lowering=False, debug=False, num_devices=NCORES)

    # ---- per-core external inputs ----
    hidT = nc.dram_tensor("hidT", [HID, TPC], BF16, kind="ExternalInput")
    wqaT = nc.dram_tensor("wqaT", [HID, QLR], BF16, kind="ExternalInput")
    wkvaT = nc.dram_tensor("wkvaT", [HID, CKW], BF16, kind="ExternalInput")
    wqbT = nc.dram_tensor("wqbT", [QLR, H * DQ], BF16, kind="ExternalInput")
    wkvbkT = nc.dram_tensor("wkvbkT", [KVLR, HPC * DN], BF16, kind="ExternalInput")
    wkvbvT = nc.dram_tensor("wkvbvT", [KVLR, HPC * DV], BF16, kind="ExternalInput")
    woT = nc.dram_tensor("woT", [H * DV, HID], BF16, kind="ExternalInput")
    cosd = nc.dram_tensor("cosd", [P, TPC], BF16, kind="ExternalInput")
    sind = nc.dram_tensor("sind", [P, TPC], BF16, kind="ExternalInput")
    masks = nc.dram_tensor("masks", [P, 4 * 512], BF16, kind="ExternalInput")
    outT = nc.dram_tensor("outT", [HID, TPC], F32, kind="ExternalOutput")

    RG = [list(range(NCORES))]

    with tile.TileContext(nc) as tc:
        with tc.tile_pool(name="dram", bufs=1, space="DRAM") as dram, \
             tc.tile_pool(name="const", bufs=1) as const, \
             tc.tile_pool(name="pwk", bufs=1) as pwk:
            latkv_in = dram.tile([CKW, TPC], BF16)
            latkv_all = dram.tile([NCORES * CKW, TPC], BF16, addr_space="Shared")
            qa_in = dram.tile([NCORES * 2 * P, TPC], BF16)   # [pe2|h0n] per pair
            qa_out = dram.tile([NCORES * 2 * P, TPC], BF16)
            qb_in = dram.tile([NCORES * P, TPC], BF16)       # h1n per pair
            qb_out = dram.tile([NCORES * P, TPC], BF16)
            oa_in = dram.tile([NCORES * DV, TPC], BF16)      # even heads out
            oa_out = dram.tile([NCORES * DV, TPC], BF16)
            ob_in = dram.tile([NCORES * DV, TPC], BF16)      # odd heads out
            ob_out = dram.tile([NCORES * DV, TPC], BF16)

            ones_col = const.tile([P, 1], BF16)
            nc.vector.memset(ones_col[:], 1.0)
            ones_row = const.tile([1, P], F32)
            nc.vector.memset(ones_row[:], 1.0)
            ones_row_b = const.tile([1, P], BF16)
            nc.vector.memset(ones_row_b[:], 1.0)
            eps_t = const.tile([1, 1], F32)
            nc.vector.memset(eps_t[:], EPS)
            cos_sb = const.tile([P, TPC], BF16)
            sin_sb = const.tile([P, TPC], BF16)
            mask_sb = const.tile([P, 4 * 512], BF16)
            wkk_sb = pwk.tile([P, 4 * WKK], BF16)
            wkv_sb = pwk.tile([P, 4 * WKK], BF16)
            ckvj_bufs = [pwk.tile([P, 4 * TPC], BF16, tag=f"ckvj{b}",
                                  name=f"ckvj{b}") for b in range(NCORES)]
            kpe_t = [pwk.tile([P, S], BF16, tag=f"kp{bb}", name=f"kp{bb}")
                     for bb in range(2)]

            # ====== phase-1 long-lived pool (wqb prefetch + normed cq) ======
            p1q_stack = ExitStack()
            p1q = p1q_stack.enter_context(tc.tile_pool(name="p1q", bufs=1))
            ps1_stack = ExitStack()
            ps1 = ps1_stack.enter_context(tc.tile_pool(name="ps1", bufs=3, space="PSUM"))
            ps1b_stack = ExitStack()
            ps1s = ps1b_stack.enter_context(tc.tile_pool(name="ps1s", bufs=1, space="PSUM"))
            ps1b = ps1b_stack.enter_context(tc.tile_pool(name="ps1b", bufs=2, space="PSUM"))

            WQBH = (H * DQ) // 2  # 1536 output dims per load half
            wqb_ch = [p1q.tile([P, WQBH], BF16, tag=f"wqb{kc}", name=f"wqb{kc}")
                      for kc in range(NQLR)]
            cqn_sb = p1q.tile([P, NQLR * TPC], BF16)

            # ============ Phase 1a/1b: token-parallel down-proj + norms ============
            p1ab_stack = ExitStack()
            p1a = p1ab_stack.enter_context(tc.tile_pool(name="p1a", bufs=1))
            p1t = p1ab_stack.enter_context(tc.tile_pool(name="p1t", bufs=2))
            p1r = p1ab_stack.enter_context(tc.tile_pool(name="p1r", bufs=1))
            p1n = p1ab_stack.enter_context(tc.tile_pool(name="p1n", bufs=1))

            hid_ch = [p1a.tile([P, TPC], BF16, tag=f"hid{kc}", name=f"hid{kc}")
                      for kc in range(NHID)]
            wkva_ch = [p1a.tile([P, CKW], BF16, tag=f"wkva{kc}", name=f"wkva{kc}")
                       for kc in range(NHID)]
            wqa_ch = [p1a.tile([P, QLR], BF16, tag=f"wqa{kc}", name=f"wqa{kc}")
                      for kc in range(NHID)]
            ckv_bf = p1a.tile([P, 4 * TPC], BF16)

            # DMA issue order = consumption order: hid+wkva feed ckv first,
            # then rope tables, then wqa (cq), then the big wqb prefetch.
            for kc in range(NHID):
                nc.sync.dma_start(hid_ch[kc][:], hidT.ap()[kc * P:(kc + 1) * P, :])
                nc.sync.dma_start(wkva_ch[kc][:], wkvaT.ap()[kc * P:(kc + 1) * P, :])
            nc.sync.dma_start(cos_sb[:], cosd.ap()[:])
            nc.sync.dma_start(sin_sb[:], sind.ap()[:])
            for kc in range(NHID):
                nc.sync.dma_start(wqa_ch[kc][:], wqaT.ap()[kc * P:(kc + 1) * P, :])
            for kc in range(NQLR):
                nc.sync.dma_start(wqb_ch[kc][:],
                                  wqbT.ap()[kc * P:(kc + 1) * P, 0:WQBH])
            for kc in range(4):
                nc.sync.dma_start(wkk_sb[:, kc * WKK:(kc + 1) * WKK],
                                  wkvbkT.ap()[kc * P:(kc + 1) * P, :])
                nc.sync.dma_start(wkv_sb[:, kc * WKK:(kc + 1) * WKK],
                                  wkvbvT.ap()[kc * P:(kc + 1) * P, :])
            nc.sync.dma_start(mask_sb[:], masks.ap()[:])

            # --- ckv joint (d-major): 4 normed blocks + k_pe block.
            # Sum-of-squares matmuls lag one block so the PE never waits.
            ssq_kv = ps1s.tile([1, TPC], F32, tag="ssqkv")
            sq_kv = []
            for m in range(4):
                ps = ps1.tile([P, TPC], F32, tag="proj")
                for kc in range(NHID):
                    nc.tensor.matmul(ps[:], wkva_ch[kc][:, m * P:(m + 1) * P],
                                     hid_ch[kc][:],
                                     start=(kc == 0), stop=(kc == NHID - 1))
                nc.scalar.copy(ckv_bf[:, m * TPC:(m + 1) * TPC], ps[:])
                sq = p1t.tile([P, TPC], BF16, tag="sq")
                nc.vector.tensor_mul(sq[:], ckv_bf[:, m * TPC:(m + 1) * TPC],
                                     ckv_bf[:, m * TPC:(m + 1) * TPC])
                sq_kv.append(sq)
                if m >= 1:
                    nc.tensor.matmul(ssq_kv[:], ones_col[:], sq_kv[m - 1][:],
                                     start=(m == 1), stop=False,
                                     skip_group_check=True)

            # k_pe block [64, TPC] (shared across heads)
            ps_pe = ps1.tile([DR, TPC], F32, tag="proj")
            for kc in range(NHID):
                nc.tensor.matmul(ps_pe[:], wkva_ch[kc][:, KVLR:CKW],
                                 hid_ch[kc][:],
                                 start=(kc == 0), stop=(kc == NHID - 1))
            nc.tensor.matmul(ssq_kv[:], ones_col[:], sq_kv[3][:],
                             start=False, stop=True, skip_group_check=True)

            # k_pe rope -> latkv_in tail
            HDR = DR // 2
            rot = p1r.tile([DR, TPC], F32, tag="rot")
            nc.scalar.mul(rot[0:HDR, :], ps_pe[HDR:DR, :], -1.0)
            nc.scalar.copy(rot[HDR:DR, :], ps_pe[0:HDR, :])
            t1 = p1r.tile([DR, TPC], F32, tag="t1")
            nc.vector.tensor_mul(t1[:], ps_pe[:], cos_sb[0:DR, :])
            nc.vector.tensor_mul(rot[:], rot[:], sin_sb[0:DR, :])
            pe_out = p1r.tile([DR, TPC], BF16, tag="peo")
            nc.vector.tensor_add(pe_out[:], t1[:], rot[:])
            nc.sync.dma_start(latkv_in[KVLR:CKW, :], pe_out[:])

            # kv norm scalars (Act + DVE) while the PE moves on to cq
            kv_norm = p1n.tile([1, TPC], F32, tag="nrm")
            nc.scalar.activation(kv_norm[:], ssq_kv[:], AF.Sqrt, bias=eps_t[:],
                                 scale=1.0 / KVLR)
            rn_kv = p1n.tile([1, TPC], F32, tag="rn")
            nc.vector.reciprocal_approx_fast(rn_kv[:], kv_norm[:])

            # --- cq blocks land unnormalized in cqn_sb; normalized in place
            ssq_q = ps1s.tile([1, TPC], F32, tag="ssqq")
            sq_q = []

            def cq_block(m):
                ps = ps1.tile([P, TPC], F32, tag="proj", name="ps")
                for kc in range(NHID):
                    nc.tensor.matmul(ps[:], wqa_ch[kc][:, m * P:(m + 1) * P],
                                     hid_ch[kc][:],
                                     start=(kc == 0), stop=(kc == NHID - 1))
                nc.scalar.copy(cqn_sb[:, m * TPC:(m + 1) * TPC], ps[:])
                sq = p1t.tile([P, TPC], BF16, tag="sq", name="sq")
                nc.vector.tensor_mul(sq[:], cqn_sb[:, m * TPC:(m + 1) * TPC],
                                     cqn_sb[:, m * TPC:(m + 1) * TPC])
                sq_q.append(sq)

            cq_block(0)
            cq_block(1)
            cq_block(2)

            # kv latents: normalize + ship + AllGather
            bkv = ps1b.tile([P, TPC], F32, tag="bc")
            nc.tensor.matmul(bkv[:], ones_row[:], rn_kv[:], start=True, stop=True)
            for m in range(4):
                lat_o = p1t.tile([P, TPC], BF16, tag="sq")
                nc.vector.tensor_mul(lat_o[:], ckv_bf[:, m * TPC:(m + 1) * TPC], bkv[:])
                nc.sync.dma_start(latkv_in[m * P:(m + 1) * P, :], lat_o[:])
            nc.gpsimd.collective_compute(
                "AllGather", mybir.AluOpType.bypass, replica_groups=RG,
                ins=[latkv_in.opt()], outs=[latkv_all.opt()])

            # --- cq blocks 3..11, ssq lagging one block ---
            nc.tensor.matmul(ssq_q[:], ones_col[:], sq_q[0][:],
                             start=True, stop=False, skip_group_check=True)
            nc.tensor.matmul(ssq_q[:], ones_col[:], sq_q[1][:],
                             start=False, stop=False, skip_group_check=True)
            for m in range(3, NQLR):
                cq_block(m)
                nc.tensor.matmul(ssq_q[:], ones_col[:], sq_q[m - 1][:],
                                 start=False, stop=False,
                                 skip_group_check=True)
            nc.tensor.matmul(ssq_q[:], ones_col[:], sq_q[NQLR - 1][:],
                             start=False, stop=True, skip_group_check=True)

            sq_norm = p1n.tile([1, TPC], F32, tag="nrm")
            nc.scalar.activation(sq_norm[:], ssq_q[:], AF.Sqrt, bias=eps_t[:],
                                 scale=1.0 / QLR)
            rn_q = p1n.tile([1, TPC], F32, tag="rn")
            nc.vector.reciprocal_approx_fast(rn_q[:], sq_norm[:])
            bq_ps = ps1b.tile([P, TPC], F32, tag="bc")
            nc.tensor.matmul(bq_ps[:], ones_row[:], rn_q[:], start=True, stop=True)
            bq = p1q.tile([P, TPC], F32)
            nc.scalar.copy(bq[:], bq_ps[:])

            ps1b_stack.close()
            p1ab_stack.close()   # frees hid/wkva/wqa/ckv sbuf

            # second half of wqb: separate pool reusing the freed phase-1a
            # space, loaded immediately (no reload seam mid q-up)
            p1wb_stack = ExitStack()
            p1wb = p1wb_stack.enter_context(tc.tile_pool(name="p1wb", bufs=1))
            wqb_chB = [p1wb.tile([P, WQBH], BF16, tag=f"wqbB{kc}",
                                 name=f"wqbB{kc}") for kc in range(NQLR)]
            for kc in range(NQLR):
                nc.sync.dma_start(wqb_chB[kc][:],
                                  wqbT.ap()[kc * P:(kc + 1) * P, WQBH:2 * WQBH])

            # gathered latents + shared rope keys prefetch into SBUF the
            # moment the AllGather lands (before the q AllToAlls hold the
            # DMA rings)
            for j in range(NCORES):
                basek = j * CKW
                for r in range(4):
                    nc.gpsimd.dma_start(
                        ckvj_bufs[j][:, r * TPC:(r + 1) * TPC],
                        latkv_all[basek + r * P: basek + (r + 1) * P, :])
                nc.gpsimd.dma_start(
                    kpe_t[j // 4][0:DR, (j % 4) * TPC:(j % 4 + 1) * TPC],
                    latkv_all[basek + KVLR: basek + CKW, :])
                nc.gpsimd.dma_start(
                    kpe_t[j // 4][DR:P, (j % 4) * TPC:(j % 4 + 1) * TPC],
                    latkv_all[basek + KVLR: basek + CKW, :])

            # ============ Phase 1c: q up-projection for ALL heads ============
            with tc.tile_pool(name="p1qt", bufs=8) as p1qt, \
                 tc.tile_pool(name="p1qr", bufs=1) as p1qr:
                wqb_cur = wqb_ch

                def qup_block(mb):
                    col = (mb % 12) * P
                    ps = ps1.tile([P, TPC], F32, tag="proj", name="ps")
                    for kc in range(NQLR):
                        nc.tensor.matmul(ps[:], wqb_cur[kc][:, col:col + P],
                                         cqn_sb[:, kc * TPC:(kc + 1) * TPC],
                                         start=(kc == 0), stop=(kc == NQLR - 1))
                    qo = p1qt.tile([P, TPC], BF16, tag="qo", name="qo")
                    if mb < 16 and mb % 2 == 0:  # pe2 block -> rope + rmsnorm
                        _rope_dual(nc, p1qr, qo, ps, cos_sb, sin_sb, "q",
                                   scale=bq)
                    else:
                        nc.vector.tensor_mul(qo[:], ps[:], bq[:])
                    if mb < 16:
                        nc.scalar.dma_start(qa_in[mb * P:(mb + 1) * P, :], qo[:])
                    else:
                        mo = mb - 16
                        nc.scalar.dma_start(qb_in[mo * P:(mo + 1) * P, :], qo[:])

                for mb in range(12):
                    qup_block(mb)
                wqb_cur = wqb_chB
                for mb in range(12, 16):
                    qup_block(mb)
                nc.gpsimd.collective_compute(
                    "AllToAll", mybir.AluOpType.bypass, replica_groups=RG,
                    ins=[qa_in.opt()], outs=[qa_out.opt()])
                for mb in range(16, 24):
                    qup_block(mb)
                nc.gpsimd.collective_compute(
                    "AllToAll", mybir.AluOpType.bypass, replica_groups=RG,
                    ins=[qb_in.opt()], outs=[qb_out.opt()])
            p1wb_stack.close()
            ps1_stack.close()
            p1q_stack.close()    # frees wqb + cqn

            # ====== phase-3 staging pool (lives to the end) ======
            p3w_stack = ExitStack()
            p3w = p3w_stack.enter_context(tc.tile_pool(name="p3w", bufs=1))

            # attention-lifetime kv tiles (split per head / per batch so the
            # first attention unit only depends on its own producers)
            atta_stack = ExitStack()
            att_a = atta_stack.enter_context(tc.tile_pool(name="att_a", bufs=1))
            knope_t = [[att_a.tile([P, S], BF16, tag=f"kn{hl}{bb}",
                                   name=f"kn{hl}{bb}") for bb in range(2)]
                       for hl in range(2)]
            v_t = [att_a.tile([P, 16 * WKK], BF16, tag=f"v{bb}", name=f"v{bb}")
                   for bb in range(2)]

            # ============ Phase 2: k/v up-projection (b0 then b1) ============
            with tc.tile_pool(name="ps2", bufs=2, space="PSUM") as ps2:
                for bbv in range(2):
                    for jj in range(4):
                        j = bbv * 4 + jj
                        ckv_j = ckvj_bufs[j]
                        for m in range(HPC):
                            ps = ps2.tile([P, TPC], F32, tag="proj")
                            for kc in range(4):
                                nc.tensor.matmul(
                                    ps[:], wkk_sb[:, kc * WKK + m * P: kc * WKK + (m + 1) * P],
                                    ckv_j[:, kc * TPC:(kc + 1) * TPC],
                                    start=(kc == 0), stop=(kc == 3))
                            nc.scalar.copy(
                                knope_t[m][bbv][:, jj * TPC:(jj + 1) * TPC], ps[:])
                        for tb in range(4):
                            ps = ps2.tile([P, WKK], F32, tag="vproj")
                            for kc in range(4):
                                nc.tensor.matmul(
                                    ps[:],
                                    ckv_j[:, kc * TPC + tb * P: kc * TPC + (tb + 1) * P],
                                    wkv_sb[:, kc * WKK:(kc + 1) * WKK],
                                    start=(kc == 0), stop=(kc == 3))
                            jb = jj * 4 + tb
                            nc.scalar.copy(
                                v_t[bbv][:, jb * WKK:(jb + 1) * WKK], ps[:])

            # w_o even-half prefetch (fires once the wqb space is released)
            woe_sb = p3w.tile([P, NCORES * HID], BF16)
            for i in range(NCORES):
                nc.sync.dma_start(woe_sb[:, i * HID:(i + 1) * HID],
                                  woT.ap()[(2 * i) * P:(2 * i + 1) * P, :])

            # q receives (fire when the AllToAlls deliver); split per head
            # and per batch for precise attention dependencies
            qpe_stack = ExitStack()
            attq = qpe_stack.enter_context(tc.tile_pool(name="attq", bufs=1))
            qnope_t = [[attq.tile([P, S], BF16, tag=f"qn{hl}{bb}",
                                  name=f"qn{hl}{bb}") for bb in range(2)]
                       for hl in range(2)]
            qpe_t = [attq.tile([P, S], BF16, tag=f"qp{bb}", name=f"qp{bb}")
                     for bb in range(2)]
            for i in range(NCORES):
                bb, col = i // 4, (i % 4) * TPC
                nc.gpsimd.dma_start(qpe_t[bb][:, col:col + TPC],
                                    qa_out[i * 2 * P: i * 2 * P + P, :])
                nc.gpsimd.dma_start(qnope_t[0][bb][:, col:col + TPC],
                                    qa_out[i * 2 * P + P: (i + 1) * 2 * P, :])

            # ============ attention (4 causal units, hl-major) ============
            with tc.tile_pool(name="att_t", bufs=2) as att_t, \
                 tc.tile_pool(name="att_e", bufs=2) as att_e, \
                 tc.tile_pool(name="att_x", bufs=3) as att_x, \
                 tc.tile_pool(name="ps_s", bufs=4, space="PSUM") as ps_s_pool, \
                 tc.tile_pool(name="ps_o", bufs=2, space="PSUM") as ps_o_pool, \
                 tc.tile_pool(name="ps_d", bufs=1, space="PSUM") as ps_d_pool:

                for u in range(4):  # hl-major: (hl, bb)
                    hl, bb = u // 2, u % 2
                    kn = knope_t[hl][bb]
                    qn = qnope_t[hl][bb]
                    chunks = [(qt, kc) for qt in range(QT_PER_B)
                              for kc in range(4 * (qt + 1))]
                    exs = {}
                    psos = {}
                    Es = {}
                    fin_pending = []

                    def emit_v(idx, chunks=chunks, exs=exs, psos=psos, Es=Es,
                               hl=hl, bb=bb):
                        qt, kc = chunks[idx]
                        nkc = 4 * (qt + 1)
                        nc.tensor.matmul(
                            psos[qt][:],
                            v_t[bb][:, kc * WKK + hl * DV: kc * WKK + (hl + 1) * DV],
                            exs.pop(idx)[:],
                            start=(kc == 0), stop=(kc == nkc - 1),
                            skip_group_check=True)
                        if kc == nkc - 1:
                            # softmax denominator: one M=1 matmul per q-tile;
                            # the broadcast + divide are deferred two chunks so
                            # the PE never waits on the reciprocal
                            ps_den = ps_d_pool.tile([1, 512], F32, tag="psd",
                                                    name="ps_den")
                            nc.tensor.matmul(ps_den[:], ones_col[:],
                                             Es.pop(qt)[:],
                                             start=True, stop=True,
                                             skip_group_check=True)
                            rec_f = att_t.tile([1, 512], F32, tag="rcf",
                                               name="rec_f")
                            nc.vector.reciprocal_approx_fast(rec_f[:], ps_den[:])
                            rec_b = att_t.tile([1, 512], BF16, tag="rcb",
                                               name="rec_b")
                            nc.vector.tensor_copy(rec_b[:], rec_f[:])
                            fin_pending.append((qt, rec_b))

                    def emit_fin():
                        qt, rec_b = fin_pending.pop(0)
                        bc = ps_s_pool.tile([P, 512], F32, tag="pss", name="bc")
                        nc.tensor.matmul(bc[:], ones_row_b[:], rec_b[:],
                                         start=True, stop=True,
                                         skip_group_check=True)
                        bc_sb = att_t.tile([P, 512], BF16, tag="bcs",
                                           name="bc_sb")
                        nc.scalar.copy(bc_sb[:], bc[:])
                        on = att_t.tile([P, 512], BF16, tag="on", name="on")
                        nc.vector.tensor_mul(on[:], psos.pop(qt)[:], bc_sb[:])
                        blk = bb * QT_PER_B + qt
                        tgt = oa_in if hl == 0 else ob_in
                        nc.gpsimd.dma_start(tgt[blk * DV:(blk + 1) * DV, :], on[:])

                    for idx, (qt, kc) in enumerate(chunks):
                        if kc == 0:
                            psos[qt] = ps_o_pool.tile([P, 512], F32, tag="pso",
                                                      name="pso")
                            Es[qt] = att_e.tile([P, 512], BF16, tag="E", name="E")
                        qoff = qt * 512
                        koff = kc * P
                        ps_sc = ps_s_pool.tile([P, 512], F32, tag="pss")
                        nc.tensor.matmul(
                            ps_sc[:], kn[:, koff: koff + P],
                            qn[:, qoff: qoff + 512],
                            start=True, stop=False)
                        nc.tensor.matmul(
                            ps_sc[:], kpe_t[bb][hl * DR: hl * DR + DR, koff: koff + P],
                            qpe_t[bb][hl * DR: hl * DR + DR, qoff: qoff + 512],
                            start=False, stop=True)
                        ex = att_x.tile([P, 512], BF16, tag="ex")
                        nc.scalar.activation(ex[:], ps_sc[:], AF.Exp)
                        if kc >= 4 * qt:
                            mi = kc - 4 * qt
                            nc.vector.tensor_mul(ex[:], ex[:],
                                                 mask_sb[:, mi * 512:(mi + 1) * 512])
                        if kc == 0:
                            nc.vector.tensor_copy(Es[qt][:], ex[:])
                        else:
                            nc.vector.tensor_add(Es[qt][:], Es[qt][:], ex[:])
                        exs[idx] = ex
                        if idx >= 2:
                            emit_v(idx - 2)
                            if kc >= 2 and fin_pending:
                                emit_fin()
                    emit_v(len(chunks) - 2)
                    emit_v(len(chunks) - 1)
                    while fin_pending:
                        emit_fin()

                    if u == 0:
                        # odd-head q receives fire once the qb AllToAll lands
                        for i in range(NCORES):
                            bbq, col = i // 4, (i % 4) * TPC
                            nc.gpsimd.dma_start(qnope_t[1][bbq][:, col:col + TPC],
                                                qb_out[i * P:(i + 1) * P, :])
                    if u == 1:  # even heads complete -> overlap with odd attention
                        nc.gpsimd.collective_compute(
                            "AllToAll", mybir.AluOpType.bypass, replica_groups=RG,
                            ins=[oa_in.opt()], outs=[oa_out.opt()])
                        # even-head receives + odd-half w_o prefetch during
                        # the odd-head attention
                        oe_sb = p3w.tile([P, NCORES * TPC], BF16)
                        for i in range(NCORES):
                            nc.sync.dma_start(oe_sb[:, i * TPC:(i + 1) * TPC],
                                              oa_out[i * P:(i + 1) * P, :])
                        woo_sb = p3w.tile([P, NCORES * HID], BF16)
                        for i in range(NCORES):
                            nc.sync.dma_start(woo_sb[:, i * HID:(i + 1) * HID],
                                              woT.ap()[(2 * i + 1) * P:(2 * i + 2) * P, :])

                nc.gpsimd.collective_compute(
                    "AllToAll", mybir.AluOpType.bypass, replica_groups=RG,
                    ins=[ob_in.opt()], outs=[ob_out.opt()])
                oo_sb = p3w.tile([P, NCORES * TPC], BF16)
                for i in range(NCORES):
                    eng = nc.sync if i % 2 == 0 else nc.scalar
                    eng.dma_start(oo_sb[:, i * TPC:(i + 1) * TPC],
                                  ob_out[i * P:(i + 1) * P, :])

            qpe_stack.close()
            atta_stack.close()

            # ============ Phase 3: two passes (pass 1 overlaps the ob AllToAll) ============
            with tc.tile_pool(name="p3p", bufs=1) as p3p, \
                 tc.tile_pool(name="p3t", bufs=3) as p3t, \
                 tc.tile_pool(name="ps3", bufs=4, space="PSUM") as ps3:
                part_sb = p3p.tile([P, NHID * TPC], F32)
                for m in range(NHID):
                    ps = ps3.tile([P, TPC], F32, tag="proj")
                    for i in range(NCORES):
                        nc.tensor.matmul(
                            ps[:], woe_sb[:, i * HID + m * P: i * HID + (m + 1) * P],
                            oe_sb[:, i * TPC:(i + 1) * TPC],
                            start=(i == 0), stop=(i == NCORES - 1))
                    nc.scalar.copy(part_sb[:, m * TPC:(m + 1) * TPC], ps[:])
                for m in range(NHID):
                    ps = ps3.tile([P, TPC], F32, tag="proj")
                    for i in range(NCORES):
                        nc.tensor.matmul(
                            ps[:], woo_sb[:, i * HID + m * P: i * HID + (m + 1) * P],
                            oo_sb[:, i * TPC:(i + 1) * TPC],
                            start=(i == 0), stop=(i == NCORES - 1))
                    ot = p3t.tile([P, TPC], F32, tag="ot")
                    nc.vector.tensor_add(ot[:], ps[:], part_sb[:, m * TPC:(m + 1) * TPC])
                    nc.sync.dma_start(outT.ap()[m * P:(m + 1) * P, :], ot[:])
            p3w_stack.close()
    nc.finalize()
    return nc


def _bf16(x):
    return np.ascontiguousarray(x.astype(ml_dtypes.bfloat16))


def _rope_tables():
    inv_freq = 1.0 / (THETA ** (np.arange(0, DR, 2, dtype=np.float64) / DR))
    t = np.arange(S, dtype=np.float64)
    freqs = np.outer(t, inv_freq)
    emb = np.concatenate((freqs, freqs), axis=-1)
    return np.cos(emb).astype(np.float32), np.sin(emb).astype(np.float32)


def prepare_inputs(hidden_states, w_qa, q_a_ln_w, w_qb, w_kva, kv_a_ln_w, w_kvb, w_o):
    hidden_states = np.asarray(hidden_states, dtype=np.float32)
    w_qa = np.asarray(w_qa, dtype=np.float32)
    q_a_ln_w = np.asarray(q_a_ln_w, dtype=np.float32)
    w_qb = np.asarray(w_qb, dtype=np.float32)
    w_kva = np.asarray(w_kva, dtype=np.float32)
    kv_a_ln_w = np.asarray(kv_a_ln_w, dtype=np.float32)
    w_kvb = np.asarray(w_kvb, dtype=np.float32)
    w_o = np.asarray(w_o, dtype=np.float32)

    flat = hidden_states.reshape(T, HID)
    cos, sin = _rope_tables()          # [S, DR]
    scale = DQ ** -0.5

    pos = np.arange(T) % S
    cos_d = cos[pos].T                 # [DR, T]
    sin_d = sin[pos].T

    kp = np.arange(P)[:, None]
    qf = np.arange(512)[None, :]
    masks = _bf16(np.concatenate(
        [(qf >= kp + P * p).astype(np.float32) for p in range(4)], axis=1))

    w_qb_eff = (w_qb * q_a_ln_w[None, :]) * scale       # [H*DQ, QLR]
    w_kvb_eff = w_kvb * kv_a_ln_w[None, :]              # [H*(DN+DV), KVLR]

    # w_qb rows permuted: block A = per pair j [h0 pe | h1 pe | h0 nope],
    # block B = per pair j [h1 nope]
    rows = []
    for j in range(NCORES):
        h0, h1 = 2 * j, 2 * j + 1
        rows.append(w_qb_eff[h0 * DQ + DN: h0 * DQ + DQ])   # h0 pe (64)
        rows.append(w_qb_eff[h1 * DQ + DN: h1 * DQ + DQ])   # h1 pe (64)
        rows.append(w_qb_eff[h0 * DQ: h0 * DQ + DN])        # h0 nope (128)
    for j in range(NCORES):
        h1 = 2 * j + 1
        rows.append(w_qb_eff[h1 * DQ: h1 * DQ + DN])        # h1 nope (128)
    wqbT_full = _bf16(np.concatenate(rows, axis=0).T)       # [QLR, 3072]

    wqaT = _bf16(w_qa.T)
    wkvaT = _bf16(w_kva.T)
    woT = _bf16(w_o.T)

    in_maps = []
    for c in range(NCORES):
        heads = [HPC * c + h for h in range(HPC)]
        krows = [w_kvb_eff[h * (DN + DV): h * (DN + DV) + DN] for h in heads]
        wkvbkT_c = _bf16(np.concatenate(krows, axis=0).T)
        vrows = [w_kvb_eff[h * (DN + DV) + DN: (h + 1) * (DN + DV)] for h in heads]
        wkvbvT_c = _bf16(np.concatenate(vrows, axis=0).T)

        tok0 = c * TPC
        cosl = cos_d[:, tok0:tok0 + TPC]
        sinl = sin_d[:, tok0:tok0 + TPC]
        in_maps.append({
            "hidT": _bf16(flat[tok0:tok0 + TPC].T),
            "wqaT": wqaT, "wkvaT": wkvaT,
            "wqbT": wqbT_full, "wkvbkT": wkvbkT_c, "wkvbvT": wkvbvT_c,
            "woT": woT,
            "cosd": _bf16(np.concatenate([cosl, cosl], axis=0)),
            "sind": _bf16(np.concatenate([sinl, sinl], axis=0)),
            "masks": masks,
        })
    return in_maps


def kernel(hidden_states, w_qa, q_a_ln_w, w_qb, w_kva, kv_a_ln_w, w_kvb, w_o,
           _trace=False):
    global _NC_CACHE
    if _NC_CACHE is None:
        _NC_CACHE = build_nc()
    nc = _NC_CACHE
    in_maps = prepare_inputs(hidden_states, w_qa, q_a_ln_w, w_qb, w_kva,
                             kv_a_ln_w, w_kvb, w_o)
    res = run_bass_kernel_spmd(nc, in_maps, core_ids=list(range(NCORES)),
                               trace=_trace)
    out = np.empty((T, HID), dtype=np.float32)
    for c in range(NCORES):
        out[c * TPC:(c + 1) * TPC] = res.results[c]["outT"].T
    if _trace:
        kernel._last_result = res
    return out.reshape(B, S, HID)
